# revision 20
# baseline (speedup 1.0000x reference)
"""Trainium2 Bass kernel for DepthwiseIIR + BatchNorm(eval) + clamp(-8, 8).

Math: the row recurrence
    y[0] = (wc+wi+wo) x[0]
    f_r  = wo f_{r-1} + x_{r-1},  f_0 = 0
    ict_r = wo ict_{r-1},         ict_0 = (wi+wo) x[0]
    y[r] = wc x[r] + (wi + wo wc) f_r + ict_r
is linear in x along H, so for each channel c the full op (including the
BN scale, folded in) is a lower-triangular matmul  Y[b,c] = T_c @ X[b,c]
with T_c built on the host from per-channel scalars:
    T[r,k] = fc wo^{r-1-k}  (k < r),  T[r,r] = wc,  T[0,0] = wc+wi+wo,
    T[r,0] += (wi+wo) wo^r  (r >= 1),  then T *= gamma/sqrt(var+eps).
The remaining epilogue is  clamp(psum + bias, -8, 8)
  = min(relu(psum + (8+bias)), 16) - 8
done as one ScalarE activation (Relu, per-partition bias) + one VectorE
tensor_scalar (min, add).

Sharding: data-parallel over channels — 8 channels per core; each core's
T blocks / bias ride along as per-core inputs, x/y stay in the natural
[B,C,H,W] layout (contraction over H = partition dim, W = free dim).
"""

import sys

import numpy as np

if "/opt/trn_rl_repo" not in sys.path:
    sys.path.insert(0, "/opt/trn_rl_repo")

B, C, H, W = 4, 64, 512, 512
EPS = 1e-3
NCORES = 8
CPC = C // NCORES  # channels per core
P = 128
NB = H // P  # 4 H-blocks
BLOCKS = [(i, j) for i in range(NB) for j in range(i + 1)]  # lower-tri block ids
NT = len(BLOCKS)  # 10


def _host_prep(w_curr, w_prev_inp, w_prev_out, gamma, beta, running_mean, running_var):
    """The scaled transfer matrix is Toeplitz plus a rank-1 column-0 term:
        T[r,c] = W[r-c] + corr[r]·[c==0]
        W[0] = wc,  W[d] = fc·wo^{d-1} (d>=1),  corr[r] = (wi+wo)·wo^r
    (the r=0 special-case y0=(wc+wi+wo)x0 is exactly corr[0]=wi+wo).
    Returns per-core:
      tm  [NCORES, CPC, P, NB*P] — lhsT for block distances d=0..NB-1:
          tm[...,k,d*P+m] = W[128d + m - k] (zero where negative)
      cr  [NCORES, CPC, 1, H]    — corr as a K=1 stationary row
      b8  [NCORES, CPC, P, 1]    — 8 + BN bias per partition
    all scaled by inv = gamma/sqrt(var+eps)."""
    wc = w_curr.astype(np.float64)
    wi = w_prev_inp.astype(np.float64)
    wo = w_prev_out.astype(np.float64)
    fc = wi + wo * wc
    inv = gamma.astype(np.float64) / np.sqrt(running_var.astype(np.float64) + EPS)
    bias = beta.astype(np.float64) - running_mean.astype(np.float64) * inv

    # W profile per channel over distances 0..H-1
    pw = wo[:, None] ** np.arange(H)[None, :]  # [C, H]: wo^p
    Wprof = np.empty((C, H))
    Wprof[:, 0] = wc
    Wprof[:, 1:] = fc[:, None] * pw[:, : H - 1]
    Wprof *= inv[:, None]
    corr = (wi + wo)[:, None] * pw * inv[:, None]  # [C, H]

    # 7 unique lhsT blocks: slices 0..3 = explicit j=0 blocks for block-row
    # i (Toeplitz + rank-1 col-0 term baked into k==0), 4..6 = shared
    # off-column blocks at distances d=0,1,2
    k = np.arange(P)
    m = np.arange(P)
    tm = np.zeros((C, P, 7 * P), np.float64)
    for i in range(NB):
        dd = 128 * i + m[None, :] - k[:, None]  # [P(k), P(m)]
        blk = Wprof[:, np.clip(dd, 0, None)] * (dd >= 0)
        blk[:, 0, :] += corr[:, 128 * i + m]
        tm[:, :, i * P : (i + 1) * P] = blk
    for d in range(NB - 1):
        dd = 128 * d + m[None, :] - k[:, None]
        tm[:, :, (4 + d) * P : (5 + d) * P] = Wprof[:, np.clip(dd, 0, None)] * (
            dd >= 0
        )
    tm = np.ascontiguousarray(tm.reshape(NCORES, CPC, P, 7 * P).astype(np.float32))

    b8 = np.broadcast_to(
        (8.0 + bias).astype(np.float32)[:, None, None], (C, P, 1)
    ).reshape(NCORES, CPC, P, 1)
    return tm, b8


def _build_program(B=B, CPC=CPC, W=W):
    import concourse.bacc as bacc
    import concourse.mybir as mybir
    from concourse.tile import TileContext

    f32 = mybir.dt.float32
    f32r = mybir.dt.float32r  # replicated-fp32 PE mode: 1 cycle/row at N>=256
    nc = bacc.Bacc("TRN2", target_bir_lowering=False, debug=False, num_devices=NCORES)
    xs = nc.dram_tensor("xs", [B, CPC, H, W], f32r, kind="ExternalInput")
    tmat = nc.dram_tensor("tmat", [CPC, P, 7 * P], f32r, kind="ExternalInput")
    biasd = nc.dram_tensor("biasd", [CPC, P, 1], f32, kind="ExternalInput")
    ys = nc.dram_tensor("ys", [B, CPC, H, W], f32, kind="ExternalOutput")

    xa = xs.ap()
    ya = ys.ap()
    ta = tmat.ap()
    ba = biasd.ap()

    pairs = [(cc, b) for cc in range(CPC) for b in range(B)]
    with TileContext(nc) as tc:
        with (
            tc.tile_pool(name="tw", bufs=CPC) as twp,
            tc.tile_pool(name="bias", bufs=CPC) as bpp,
            tc.tile_pool(name="xt", bufs=4) as xp,
            tc.tile_pool(name="ot", bufs=3) as opp,
            tc.tile_pool(name="ps", bufs=8, space="PSUM") as pp,
        ):
            # prologue: all per-channel weights/bias up front so channel
            # switches never stall the load stream
            tws, bts = [], []
            for cc in range(CPC):
                tw = twp.tile([P, 7 * P], f32r, tag="tw")
                nc.sync.dma_start(out=tw, in_=ta[cc])
                bt = bpp.tile([P, 1], f32, tag="bt")
                nc.sync.dma_start(out=bt, in_=ba[cc])
                tws.append(tw)
                bts.append(bt)

            xts = {}

            def load(p):
                cc, b = pairs[p]
                xt = xp.tile([P, NB, W], f32r, tag="xt")
                # whole [H, W] image for this (b, c) as one 1 MiB DMA:
                # partition p holds rows {p, 128+p, 256+p, 384+p}
                nc.sync.dma_start(
                    out=xt, in_=xa[b, cc].rearrange("(j p) w -> p j w", p=P)
                )
                xts[p] = xt

            load(0)
            load(1)
            for p, (cc, b) in enumerate(pairs):
                if p + 2 < len(pairs):
                    load(p + 2)
                xt = xts.pop(p)
                tw, bt = tws[cc], bts[cc]
                ot = opp.tile([P, NB, W], f32, tag="ot")
                for i in range(NB):
                    ps = pp.tile([P, W], f32, tag="ps")
                    for j in range(i + 1):
                        t = i if j == 0 else 4 + (i - j)
                        nc.tensor.matmul(
                            ps,
                            tw[:, t * P : (t + 1) * P],
                            xt[:, j],
                            start=(j == 0),
                            stop=(j == i),
                        )
                    nc.scalar.activation(
                        ot[:, i],
                        ps,
                        mybir.ActivationFunctionType.Relu,
                        bias=bt[:, 0:1],
                        scale=1.0,
                    )
                    nc.vector.tensor_scalar(
                        out=ot[:, i],
                        in0=ot[:, i],
                        scalar1=16.0,
                        scalar2=-8.0,
                        op0=mybir.AluOpType.min,
                        op1=mybir.AluOpType.add,
                    )
                # stores ride SWDGE (gpsimd) so their sem-waits can't
                # head-of-line block the HWDGE load stream
                nc.gpsimd.dma_start(
                    out=ya[b, cc].rearrange("(i p) w -> p i w", p=P), in_=ot
                )
    nc.compile()
    return nc


def _make_in_maps(x, tm, b8):
    return [
        {
            "xs": np.ascontiguousarray(x[:, k * CPC : (k + 1) * CPC]),
            "tmat": tm[k],
            "biasd": b8[k],
        }
        for k in range(NCORES)
    ]


def _run(inputs, trace=False):
    from concourse import bass_utils

    x = np.ascontiguousarray(np.asarray(inputs["x"], np.float32))
    tm, b8 = _host_prep(
        np.asarray(inputs["w_curr"]),
        np.asarray(inputs["w_prev_inp"]),
        np.asarray(inputs["w_prev_out"]),
        np.asarray(inputs["gamma"]),
        np.asarray(inputs["beta"]),
        np.asarray(inputs["running_mean"]),
        np.asarray(inputs["running_var"]),
    )
    nc = _build_program()
    res = bass_utils.run_bass_kernel_spmd(
        nc, _make_in_maps(x, tm, b8), core_ids=list(range(NCORES)), trace=trace
    )
    y = np.empty((B, C, H, W), np.float32)
    for k in range(NCORES):
        y[:, k * CPC : (k + 1) * CPC] = res.results[k]["ys"]
    return y, res


def kernel(**inputs):
    y, _ = _run(inputs, trace=False)
    return y


# revision 21
# speedup vs baseline: 1.0005x; 1.0005x over previous
"""Trainium2 Bass kernel for DepthwiseIIR + BatchNorm(eval) + clamp(-8, 8).

Math: the row recurrence
    y[0] = (wc+wi+wo) x[0]
    f_r  = wo f_{r-1} + x_{r-1},  f_0 = 0
    ict_r = wo ict_{r-1},         ict_0 = (wi+wo) x[0]
    y[r] = wc x[r] + (wi + wo wc) f_r + ict_r
is linear in x along H, so for each channel c the full op (including the
BN scale, folded in) is a lower-triangular matmul  Y[b,c] = T_c @ X[b,c]
with T_c built on the host from per-channel scalars:
    T[r,k] = fc wo^{r-1-k}  (k < r),  T[r,r] = wc,  T[0,0] = wc+wi+wo,
    T[r,0] += (wi+wo) wo^r  (r >= 1),  then T *= gamma/sqrt(var+eps).
The remaining epilogue is  clamp(psum + bias, -8, 8)
  = min(relu(psum + (8+bias)), 16) - 8
done as one ScalarE activation (Relu, per-partition bias) + one VectorE
tensor_scalar (min, add).

Sharding: data-parallel over channels — 8 channels per core; each core's
T blocks / bias ride along as per-core inputs, x/y stay in the natural
[B,C,H,W] layout (contraction over H = partition dim, W = free dim).
"""

import sys

import numpy as np

if "/opt/trn_rl_repo" not in sys.path:
    sys.path.insert(0, "/opt/trn_rl_repo")

B, C, H, W = 4, 64, 512, 512
EPS = 1e-3
NCORES = 8
CPC = C // NCORES  # channels per core
P = 128
NB = H // P  # 4 H-blocks
BLOCKS = [(i, j) for i in range(NB) for j in range(i + 1)]  # lower-tri block ids
NT = len(BLOCKS)  # 10


def _host_prep(w_curr, w_prev_inp, w_prev_out, gamma, beta, running_mean, running_var):
    """The scaled transfer matrix is Toeplitz plus a rank-1 column-0 term:
        T[r,c] = W[r-c] + corr[r]·[c==0]
        W[0] = wc,  W[d] = fc·wo^{d-1} (d>=1),  corr[r] = (wi+wo)·wo^r
    (the r=0 special-case y0=(wc+wi+wo)x0 is exactly corr[0]=wi+wo).
    Returns per-core:
      tm  [NCORES, CPC, P, NB*P] — lhsT for block distances d=0..NB-1:
          tm[...,k,d*P+m] = W[128d + m - k] (zero where negative)
      cr  [NCORES, CPC, 1, H]    — corr as a K=1 stationary row
      b8  [NCORES, CPC, P, 1]    — 8 + BN bias per partition
    all scaled by inv = gamma/sqrt(var+eps)."""
    wc = w_curr.astype(np.float64)
    wi = w_prev_inp.astype(np.float64)
    wo = w_prev_out.astype(np.float64)
    fc = wi + wo * wc
    inv = gamma.astype(np.float64) / np.sqrt(running_var.astype(np.float64) + EPS)
    bias = beta.astype(np.float64) - running_mean.astype(np.float64) * inv

    # W profile per channel over distances 0..H-1
    pw = wo[:, None] ** np.arange(H)[None, :]  # [C, H]: wo^p
    Wprof = np.empty((C, H))
    Wprof[:, 0] = wc
    Wprof[:, 1:] = fc[:, None] * pw[:, : H - 1]
    Wprof *= inv[:, None]
    corr = (wi + wo)[:, None] * pw * inv[:, None]  # [C, H]

    # 7 unique lhsT blocks: slices 0..3 = explicit j=0 blocks for block-row
    # i (Toeplitz + rank-1 col-0 term baked into k==0), 4..6 = shared
    # off-column blocks at distances d=0,1,2
    k = np.arange(P)
    m = np.arange(P)
    tm = np.zeros((C, P, 7 * P), np.float64)
    for i in range(NB):
        dd = 128 * i + m[None, :] - k[:, None]  # [P(k), P(m)]
        blk = Wprof[:, np.clip(dd, 0, None)] * (dd >= 0)
        blk[:, 0, :] += corr[:, 128 * i + m]
        tm[:, :, i * P : (i + 1) * P] = blk
    for d in range(NB - 1):
        dd = 128 * d + m[None, :] - k[:, None]
        tm[:, :, (4 + d) * P : (5 + d) * P] = Wprof[:, np.clip(dd, 0, None)] * (
            dd >= 0
        )
    tm = np.ascontiguousarray(tm.reshape(NCORES, CPC, P, 7 * P).astype(np.float32))

    b8 = np.broadcast_to(
        (8.0 + bias).astype(np.float32)[:, None, None], (C, P, 1)
    ).reshape(NCORES, CPC, P, 1)
    return tm, b8


def _build_program(B=B, CPC=CPC, W=W):
    import concourse.bacc as bacc
    import concourse.mybir as mybir
    from concourse.tile import TileContext

    f32 = mybir.dt.float32
    f32r = mybir.dt.float32r  # replicated-fp32 PE mode: 1 cycle/row at N>=256
    nc = bacc.Bacc("TRN2", target_bir_lowering=False, debug=False, num_devices=NCORES)
    xs = nc.dram_tensor("xs", [B, CPC, H, W], f32r, kind="ExternalInput")
    tmat = nc.dram_tensor("tmat", [CPC, P, 7 * P], f32r, kind="ExternalInput")
    biasd = nc.dram_tensor("biasd", [CPC, P, 1], f32, kind="ExternalInput")
    ys = nc.dram_tensor("ys", [B, CPC, H, W], f32, kind="ExternalOutput")

    xa = xs.ap()
    ya = ys.ap()
    ta = tmat.ap()
    ba = biasd.ap()

    pairs = [(cc, b) for cc in range(CPC) for b in range(B)]
    with TileContext(nc) as tc:
        with (
            tc.tile_pool(name="tw", bufs=CPC) as twp,
            tc.tile_pool(name="bias", bufs=CPC) as bpp,
            tc.tile_pool(name="xt", bufs=5) as xp,
            tc.tile_pool(name="ot", bufs=4) as opp,
            tc.tile_pool(name="ps", bufs=8, space="PSUM") as pp,
        ):
            # prologue: all per-channel weights/bias up front so channel
            # switches never stall the load stream
            tws, bts = [], []
            for cc in range(CPC):
                tw = twp.tile([P, 7 * P], f32r, tag="tw")
                nc.sync.dma_start(out=tw, in_=ta[cc])
                bt = bpp.tile([P, 1], f32, tag="bt")
                nc.sync.dma_start(out=bt, in_=ba[cc])
                tws.append(tw)
                bts.append(bt)

            xts = {}

            def load(p):
                cc, b = pairs[p]
                xt = xp.tile([P, NB, W], f32r, tag="xt")
                # whole [H, W] image for this (b, c) as one 1 MiB DMA:
                # partition p holds rows {p, 128+p, 256+p, 384+p}
                nc.sync.dma_start(
                    out=xt, in_=xa[b, cc].rearrange("(j p) w -> p j w", p=P)
                )
                xts[p] = xt

            load(0)
            load(1)
            for p, (cc, b) in enumerate(pairs):
                if p + 2 < len(pairs):
                    load(p + 2)
                xt = xts.pop(p)
                tw, bt = tws[cc], bts[cc]
                ot = opp.tile([P, NB, W], f32, tag="ot")
                for i in range(NB):
                    ps = pp.tile([P, W], f32, tag="ps")
                    for j in range(i + 1):
                        t = i if j == 0 else 4 + (i - j)
                        nc.tensor.matmul(
                            ps,
                            tw[:, t * P : (t + 1) * P],
                            xt[:, j],
                            start=(j == 0),
                            stop=(j == i),
                        )
                    nc.scalar.activation(
                        ot[:, i],
                        ps,
                        mybir.ActivationFunctionType.Relu,
                        bias=bt[:, 0:1],
                        scale=1.0,
                    )
                    nc.vector.tensor_scalar(
                        out=ot[:, i],
                        in0=ot[:, i],
                        scalar1=16.0,
                        scalar2=-8.0,
                        op0=mybir.AluOpType.min,
                        op1=mybir.AluOpType.add,
                    )
                # stores ride SWDGE (gpsimd) so their sem-waits can't
                # head-of-line block the HWDGE load stream
                nc.gpsimd.dma_start(
                    out=ya[b, cc].rearrange("(i p) w -> p i w", p=P), in_=ot
                )
    nc.compile()
    return nc


def _make_in_maps(x, tm, b8):
    return [
        {
            "xs": np.ascontiguousarray(x[:, k * CPC : (k + 1) * CPC]),
            "tmat": tm[k],
            "biasd": b8[k],
        }
        for k in range(NCORES)
    ]


def _run(inputs, trace=False):
    from concourse import bass_utils

    x = np.ascontiguousarray(np.asarray(inputs["x"], np.float32))
    tm, b8 = _host_prep(
        np.asarray(inputs["w_curr"]),
        np.asarray(inputs["w_prev_inp"]),
        np.asarray(inputs["w_prev_out"]),
        np.asarray(inputs["gamma"]),
        np.asarray(inputs["beta"]),
        np.asarray(inputs["running_mean"]),
        np.asarray(inputs["running_var"]),
    )
    nc = _build_program()
    res = bass_utils.run_bass_kernel_spmd(
        nc, _make_in_maps(x, tm, b8), core_ids=list(range(NCORES)), trace=trace
    )
    y = np.empty((B, C, H, W), np.float32)
    for k in range(NCORES):
        y[:, k * CPC : (k + 1) * CPC] = res.results[k]["ys"]
    return y, res


def kernel(**inputs):
    y, _ = _run(inputs, trace=False)
    return y


# revision 32
# speedup vs baseline: 1.0246x; 1.0241x over previous
"""Trainium2 Bass kernel for DepthwiseIIR + BatchNorm(eval) + clamp(-8, 8).

Math: the row recurrence
    y[0] = (wc+wi+wo) x[0]
    f_r  = wo f_{r-1} + x_{r-1},  f_0 = 0
    ict_r = wo ict_{r-1},         ict_0 = (wi+wo) x[0]
    y[r] = wc x[r] + (wi + wo wc) f_r + ict_r
is linear in x along H, so for each channel c the full op (including the
BN scale, folded in) is a lower-triangular matmul  Y[b,c] = T_c @ X[b,c]
with T_c built on the host from per-channel scalars:
    T[r,k] = fc wo^{r-1-k}  (k < r),  T[r,r] = wc,  T[0,0] = wc+wi+wo,
    T[r,0] += (wi+wo) wo^r  (r >= 1),  then T *= gamma/sqrt(var+eps).
The remaining epilogue is  clamp(psum + bias, -8, 8)
  = min(relu(psum + (8+bias)), 16) - 8
done as one ScalarE activation (Relu, per-partition bias) + one VectorE
tensor_scalar (min, add).

Sharding: data-parallel over channels — 8 channels per core; each core's
T blocks / bias ride along as per-core inputs, x/y stay in the natural
[B,C,H,W] layout (contraction over H = partition dim, W = free dim).
"""

import sys

import numpy as np

if "/opt/trn_rl_repo" not in sys.path:
    sys.path.insert(0, "/opt/trn_rl_repo")

B, C, H, W = 4, 64, 512, 512
EPS = 1e-3
NCORES = 8
CPC = C // NCORES  # channels per core
P = 128
NB = H // P  # 4 H-blocks
BLOCKS = [(i, j) for i in range(NB) for j in range(i + 1)]  # lower-tri block ids
NT = len(BLOCKS)  # 10


def _host_prep(w_curr, w_prev_inp, w_prev_out, gamma, beta, running_mean, running_var):
    """The scaled transfer matrix is Toeplitz plus a rank-1 column-0 term:
        T[r,c] = W[r-c] + corr[r]·[c==0]
        W[0] = wc,  W[d] = fc·wo^{d-1} (d>=1),  corr[r] = (wi+wo)·wo^r
    (the r=0 special-case y0=(wc+wi+wo)x0 is exactly corr[0]=wi+wo).
    Returns per-core:
      tm  [NCORES, CPC, P, NB*P] — lhsT for block distances d=0..NB-1:
          tm[...,k,d*P+m] = W[128d + m - k] (zero where negative)
      cr  [NCORES, CPC, 1, H]    — corr as a K=1 stationary row
      b8  [NCORES, CPC, P, 1]    — 8 + BN bias per partition
    all scaled by inv = gamma/sqrt(var+eps)."""
    wc = w_curr.astype(np.float64)
    wi = w_prev_inp.astype(np.float64)
    wo = w_prev_out.astype(np.float64)
    fc = wi + wo * wc
    inv = gamma.astype(np.float64) / np.sqrt(running_var.astype(np.float64) + EPS)
    bias = beta.astype(np.float64) - running_mean.astype(np.float64) * inv

    # W profile per channel over distances 0..H-1
    pw = wo[:, None] ** np.arange(H)[None, :]  # [C, H]: wo^p
    Wprof = np.empty((C, H))
    Wprof[:, 0] = wc
    Wprof[:, 1:] = fc[:, None] * pw[:, : H - 1]
    Wprof *= inv[:, None]
    corr = (wi + wo)[:, None] * pw * inv[:, None]  # [C, H]

    # Ship only the NB shared Toeplitz blocks (distances d=0..3) plus the
    # column-0 row of T' (j0r = Wprof + corr); the j=0 blocks are
    # reconstructed on-chip as copy(D_i) with partition 0 patched to j0r.
    k = np.arange(P)
    m = np.arange(P)
    tm = np.zeros((C, P, NB * P), np.float64)
    for d in range(NB):
        dd = 128 * d + m[None, :] - k[:, None]  # [P(k), P(m)]
        tm[:, :, d * P : (d + 1) * P] = Wprof[:, np.clip(dd, 0, None)] * (dd >= 0)
    tm = np.ascontiguousarray(tm.reshape(NCORES, CPC, P, NB * P).astype(np.float32))

    j0r = np.ascontiguousarray(
        (Wprof + corr).astype(np.float32).reshape(NCORES, 1, CPC * H)
    )

    b8 = np.ascontiguousarray(
        np.broadcast_to((8.0 + bias).astype(np.float32).reshape(NCORES, 1, CPC), (NCORES, P, CPC))
    )
    return tm, j0r, b8


def _build_program(B=B, CPC=CPC, W=W):
    import concourse.bacc as bacc
    import concourse.mybir as mybir
    from concourse.tile import TileContext

    f32 = mybir.dt.float32
    f32r = mybir.dt.float32r  # replicated-fp32 PE mode: 1 cycle/row at N>=256
    nc = bacc.Bacc("TRN2", target_bir_lowering=False, debug=False, num_devices=NCORES)
    xs = nc.dram_tensor("xs", [B, CPC, H, W], f32r, kind="ExternalInput")
    tmat = nc.dram_tensor("tmat", [CPC, P, NB * P], f32r, kind="ExternalInput")
    j0rd = nc.dram_tensor("j0rd", [1, CPC * H], f32r, kind="ExternalInput")
    biasd = nc.dram_tensor("biasd", [P, CPC], f32, kind="ExternalInput")
    ys = nc.dram_tensor("ys", [B, CPC, H, W], f32, kind="ExternalOutput")

    xa = xs.ap()
    ya = ys.ap()

    # group two adjacent channels (same batch) per load: their [H, W]
    # images are contiguous in DRAM, so one 2 MiB DMA stays a 3-dim AP
    groups = [
        [(cc0, b), (cc0 + 1, b)]
        for cc0 in range(0, CPC, 2)
        for b in range(B)
    ]
    with TileContext(nc) as tc:
        with (
            tc.tile_pool(name="tw", bufs=1) as twp,
            tc.tile_pool(name="xt", bufs=4) as xp,
            tc.tile_pool(name="ot", bufs=4) as opp,
            tc.tile_pool(name="ps", bufs=8, space="PSUM") as pp,
        ):
            # prologue: ONE DMA each for the Toeplitz blocks, the column-0
            # rows, and the biases; then reconstruct the per-channel j=0
            # blocks on-chip (copy D_i, patch partition 0 with j0r)
            tw = twp.tile([P, CPC, NB * P], f32r, tag="tw")
            nc.sync.dma_start(out=tw, in_=tmat.ap().rearrange("c p m -> p c m"))
            j0t = twp.tile([1, CPC * H], f32r, tag="j0t")
            nc.sync.dma_start(out=j0t, in_=j0rd.ap())
            bt = twp.tile([P, CPC], f32, tag="bt")
            nc.sync.dma_start(out=bt, in_=biasd.ap())
            ptw = twp.tile([P, CPC, NB * P], f32r, tag="ptw")
            for cc in range(CPC):
                nc.vector.tensor_copy(out=ptw[:, cc], in_=tw[:, cc])
                nc.vector.tensor_copy(
                    out=ptw[0:1, cc], in_=j0t[0:1, cc * H : (cc + 1) * H]
                )

            xts = {}

            def load(g):
                cc0, b = groups[g][0]
                xt = xp.tile([P, 2, NB, W], f32r, tag="xt")
                # two adjacent channels' [H, W] images as one 2 MiB DMA:
                # partition p holds rows {p, 128+p, 256+p, 384+p}
                nc.sync.dma_start(
                    out=xt,
                    in_=xa[b, cc0 : cc0 + 2].rearrange("c (j p) w -> p c j w", p=P),
                )
                xts[g] = xt

            load(0)
            load(1)
            for g, grp in enumerate(groups):
                if g + 2 < len(groups):
                    load(g + 2)
                xt = xts.pop(g)
                for ci, (cc, b) in enumerate(grp):
                    ot = opp.tile([P, NB, W], f32, tag="ot")
                    for i in range(NB):
                        ps = pp.tile([P, W], f32, tag="ps")
                        for j in range(i + 1):
                            lhsT = (
                                ptw[:, cc, i * P : (i + 1) * P]
                                if j == 0
                                else tw[:, cc, (i - j) * P : (i - j + 1) * P]
                            )
                            nc.tensor.matmul(
                                ps,
                                lhsT,
                                xt[:, ci, j],
                                start=(j == 0),
                                stop=(j == i),
                            )
                        nc.scalar.activation(
                            ot[:, i],
                            ps,
                            mybir.ActivationFunctionType.Relu,
                            bias=bt[:, cc : cc + 1],
                            scale=1.0,
                        )
                        nc.vector.tensor_scalar(
                            out=ot[:, i],
                            in0=ot[:, i],
                            scalar1=16.0,
                            scalar2=-8.0,
                            op0=mybir.AluOpType.min,
                            op1=mybir.AluOpType.add,
                        )
                    # stores ride SWDGE (gpsimd) so their sem-waits can't
                    # head-of-line block the HWDGE load stream
                    nc.gpsimd.dma_start(
                        out=ya[b, cc].rearrange("(i p) w -> p i w", p=P), in_=ot
                    )
    nc.compile()
    return nc


def _make_in_maps(x, tm, j0r, b8):
    return [
        {
            "xs": np.ascontiguousarray(x[:, k * CPC : (k + 1) * CPC]),
            "tmat": tm[k],
            "j0rd": j0r[k],
            "biasd": b8[k],
        }
        for k in range(NCORES)
    ]


def _run(inputs, trace=False):
    from concourse import bass_utils

    x = np.ascontiguousarray(np.asarray(inputs["x"], np.float32))
    tm, j0r, b8 = _host_prep(
        np.asarray(inputs["w_curr"]),
        np.asarray(inputs["w_prev_inp"]),
        np.asarray(inputs["w_prev_out"]),
        np.asarray(inputs["gamma"]),
        np.asarray(inputs["beta"]),
        np.asarray(inputs["running_mean"]),
        np.asarray(inputs["running_var"]),
    )
    nc = _build_program()
    res = bass_utils.run_bass_kernel_spmd(
        nc, _make_in_maps(x, tm, j0r, b8), core_ids=list(range(NCORES)), trace=trace
    )
    y = np.empty((B, C, H, W), np.float32)
    for k in range(NCORES):
        y[:, k * CPC : (k + 1) * CPC] = res.results[k]["ys"]
    return y, res


def kernel(**inputs):
    y, _ = _run(inputs, trace=False)
    return y
